# revision 4
# baseline (speedup 1.0000x reference)
"""Conv2D (N=32, Cin=128, 56x56 -> Cout=256, 3x3, pad 1, stride 1) on 8 Trainium2
NeuronCores.

Strategy: data-parallel over batch (4 images per core), conv lowered to 9
shifted matmuls (one per filter tap) accumulating in PSUM over the
Cin=128-partition contraction dim.  Cout=256 is handled as 2 halves of 128
output partitions.

v2 restructure vs baseline:
- Tap-OUTER loop order: for each (image, cout-half), the 9 taps walk all 7
  row-blocks with the SAME stationary weight, so one LDWEIGHTS serves 7
  matmuls (72 LDWs total instead of 504) and the weight-load is never on
  the critical path.  7 PSUM banks hold the 7 row-block accumulators.
- Whole-image input DMA in 2 chunks (rows 0-34 / 32-58) instead of 11-row
  strips: kills the 37% halo re-transfer and uses large contiguous
  descriptors.
- Output written PADDED (58-wide rows, 464 per row-block) to a DRAM scratch
  laid out [img, cout, rb, 464] so each store is one contiguous
  1856B-per-partition descriptor; the 2 garbage columns per row are sliced
  out on the host.  Output DMAs are spread over the gpsimd and sync queues.
- PSUM drains (bias-add + copy to SBUF) alternate between the Vector engine
  (tensor_scalar_add) and the Scalar engine (activation Identity with bias)
  so drains never serialize behind a single engine.

Matmul dtype fp32r: the PE's single-pass fp32 mode streaming at 1
column/cycle when warm (464 cols = ~193 ns).  Inputs are pre-rounded to the
fp32r grid on the host; measured scale-rel error vs the fp32 reference is
~1.5e-4.
"""

import os
import sys

import numpy as np

sys.path.insert(0, "/opt/trn_rl_repo")

import concourse.tile as tile
from concourse import bacc, mybir

N, CIN, H, W = 32, 128, 56, 56
COUT, KH, KW = 256, 3, 3
NCORES = 8
NPER = N // NCORES  # images per core
HP, WP = H + 3, W + 2  # padded spatial dims (1 top + 2 bottom, 1 left + 1 right)
FLAT = HP * WP  # 3422 padded pixels per image per cin
RB = 8  # output rows per PSUM chunk
NRB = H // RB  # 7 row-blocks per image
CHUNK = RB * WP  # 464 <= 512 fp32 PSUM bank limit
NTAP = KH * KW

# input tile split: xa covers padded rows [0, 35) for rbs 0-3,
# xb covers padded rows [32, 59) for rbs 4-6
XA_ROWS = 35
XB_ROW0 = 32
XA_LEN = XA_ROWS * WP  # 2030
XB_LEN = (HP - XB_ROW0) * WP  # 27*58 = 1566

MM_MODE = os.environ.get("CONV_MM_MODE", "fp32r")

_CACHE = {}


def _build(mm_mode):
    f32 = mybir.dt.float32
    in_dt = {
        "fp32": f32,
        "fp32r": mybir.dt.float32r,
        "bf16": mybir.dt.bfloat16,
    }[mm_mode]

    nc = bacc.Bacc(None, target_bir_lowering=False)
    xp_d = nc.declare_dram_parameter("xp", [NPER, CIN, FLAT], in_dt, isOutput=False)
    w_d = nc.declare_dram_parameter("w", [CIN, NTAP, COUT], in_dt, isOutput=False)
    b_d = nc.declare_dram_parameter("b", [CIN, 2], f32, isOutput=False)
    # padded output: [img, cout, rb, 8*58]; garbage cols sliced on host
    y_d = nc.declare_dram_parameter("y", [NPER, COUT, NRB, CHUNK], f32, isOutput=True)

    with tile.TileContext(nc) as tc:
        with (
            tc.tile_pool(name="xa", bufs=2) as xapool,
            tc.tile_pool(name="xb", bufs=2) as xbpool,
            tc.tile_pool(name="wgt", bufs=1) as wpool,
            tc.tile_pool(name="bias", bufs=1) as bpool,
            tc.tile_pool(name="out", bufs=8) as opool,
            tc.tile_pool(name="ps", bufs=8, space="PSUM") as pspool,
        ):
            # one tile per tap so an LDW only waits on its own tap's DMA
            w_taps = []
            for tap in range(NTAP):
                wt = wpool.tile([CIN, COUT], in_dt, tag=f"w{tap}")
                nc.scalar.dma_start(out=wt[:], in_=w_d[:, tap, :])
                w_taps.append(wt)
            b_sb = bpool.tile([CIN, 2], f32)
            nc.scalar.dma_start(out=b_sb[:], in_=b_d[:, :])

            for i in range(NPER):
                xa = xapool.tile([CIN, XA_LEN], in_dt, tag="xa")
                nc.sync.dma_start(out=xa[:], in_=xp_d[i, :, 0:XA_LEN])
                xb = xbpool.tile([CIN, XB_LEN], in_dt, tag="xb")
                nc.sync.dma_start(
                    out=xb[:], in_=xp_d[i, :, XB_ROW0 * WP : XB_ROW0 * WP + XB_LEN]
                )
                for half in range(2):
                    ps_tiles = [
                        pspool.tile(
                            [128, CHUNK], f32, name=f"ps_{i}_{half}_{rb}", tag="ps"
                        )
                        for rb in range(NRB)
                    ]
                    for tap in range(NTAP):
                        kh, kw = divmod(tap, KW)
                        wsl = w_taps[tap][:, half * 128 : half * 128 + 128]
                        for rb in range(NRB):
                            off = (rb * RB + kh) * WP + kw
                            if rb < 4:
                                src = xa[:, off : off + CHUNK]
                            else:
                                o = off - XB_ROW0 * WP
                                src = xb[:, o : o + CHUNK]
                            nc.tensor.matmul(
                                ps_tiles[rb][:],
                                wsl,
                                src,
                                start=(tap == 0),
                                stop=(tap == NTAP - 1),
                            )
                    for rb in range(NRB):
                        ot = opool.tile([128, CHUNK], f32)
                        if rb % 2 == 0:
                            nc.vector.tensor_scalar_add(
                                ot[:], ps_tiles[rb][:], b_sb[:, half : half + 1]
                            )
                            dma_eng = nc.gpsimd
                        else:
                            nc.scalar.activation(
                                ot[:],
                                ps_tiles[rb][:],
                                mybir.ActivationFunctionType.Identity,
                                bias=b_sb[:, half : half + 1],
                            )
                            dma_eng = nc.sync
                        dma_eng.dma_start(
                            out=y_d[i, half * 128 : half * 128 + 128, rb, :],
                            in_=ot[:],
                        )
    nc.finalize()
    return nc


def get_nc(mm_mode=None):
    mm_mode = mm_mode or MM_MODE
    if mm_mode not in _CACHE:
        _CACHE[mm_mode] = _build(mm_mode)
    return _CACHE[mm_mode]


def _round_fp32r(a):
    """Round fp32 array to the fp32r grid (8-bit exp, 11-bit mantissa, top 20
    bits of the word) with round-to-nearest so the PE's truncation of the low
    12 bits lands on the nearest representable value."""
    u = np.ascontiguousarray(a, np.float32).view(np.uint32)
    u = u + 0x7FF + ((u >> 12) & 1)
    u &= np.uint32(0xFFFFF000)
    return u.view(np.float32)


def prep_inputs(x, weight, bias, mm_mode=None):
    """Host-side staging: zero-pad x to 59x58 and flatten, retile weights to
    [cin, tap, cout], split per-core input maps."""
    mm_mode = mm_mode or MM_MODE
    x = np.asarray(x, np.float32)
    weight = np.asarray(weight, np.float32)
    bias = np.asarray(bias, np.float32)

    xp = np.zeros((N, CIN, HP, WP), np.float32)
    xp[:, :, 1 : H + 1, 1 : W + 1] = x
    # [cout, cin, kh, kw] -> [cin, tap, cout]
    w_prep = np.ascontiguousarray(weight.transpose(1, 2, 3, 0).reshape(CIN, NTAP, COUT))
    if mm_mode == "bf16":
        import ml_dtypes

        xp = xp.astype(ml_dtypes.bfloat16)
        w_prep = w_prep.astype(ml_dtypes.bfloat16)
    elif mm_mode == "fp32r":
        xp = _round_fp32r(xp)
        w_prep = _round_fp32r(w_prep)
    xp = xp.reshape(N, CIN, FLAT)
    b_prep = np.ascontiguousarray(bias.reshape(2, 128).T.astype(np.float32))

    return [
        {
            "xp": np.ascontiguousarray(xp[c * NPER : (c + 1) * NPER]),
            "w": w_prep,
            "b": b_prep,
        }
        for c in range(NCORES)
    ]


def _unpad_output(y_pad):
    """[NPER, COUT, NRB, 464] padded rows -> [NPER, COUT, 56, 56]."""
    y = y_pad.reshape(NPER, COUT, NRB, RB, WP)[:, :, :, :, :W]
    return np.ascontiguousarray(y.reshape(NPER, COUT, H, W))


def kernel(x, weight, bias, mm_mode=None, trace=False, tmpdir=None):
    from concourse.bass_utils import run_bass_kernel_spmd

    nc = get_nc(mm_mode)
    in_maps = prep_inputs(x, weight, bias, mm_mode)
    res = run_bass_kernel_spmd(
        nc, in_maps, list(range(NCORES)), trace=trace, tmpdir=tmpdir
    )
    out = np.concatenate([_unpad_output(r["y"]) for r in res.results], axis=0)
    if trace:
        kernel.last_results = res
    return out


# revision 8
# speedup vs baseline: 1.1328x; 1.1328x over previous
"""Conv2D (N=32, Cin=128, 56x56 -> Cout=256, 3x3, pad 1, stride 1) on 8 Trainium2
NeuronCores.

Strategy: data-parallel over batch (4 images per core), conv lowered to 9
shifted matmuls (one per filter tap) accumulating in PSUM over the
Cin=128-partition contraction dim.  Cout=256 is handled as 2 halves of 128
output partitions.

v2 restructure vs baseline:
- Tap-OUTER loop order: for each (image, cout-half), the 9 taps walk all 7
  row-blocks with the SAME stationary weight, so one LDWEIGHTS serves 7
  matmuls (72 LDWs total instead of 504) and the weight-load is never on
  the critical path.  7 PSUM banks hold the 7 row-block accumulators.
- Whole-image input DMA in 2 chunks (rows 0-34 / 32-58) instead of 11-row
  strips: kills the 37% halo re-transfer and uses large contiguous
  descriptors.
- Output written PADDED (58-wide rows, 464 per row-block) to a DRAM scratch
  laid out [img, cout, rb, 464] so each store is one contiguous
  1856B-per-partition descriptor; the 2 garbage columns per row are sliced
  out on the host.  Output DMAs are spread over the gpsimd and sync queues.
- PSUM drains (bias-add + copy to SBUF) alternate between the Vector engine
  (tensor_scalar_add) and the Scalar engine (activation Identity with bias)
  so drains never serialize behind a single engine.

Matmul dtype fp32r: the PE's single-pass fp32 mode streaming at 1
column/cycle when warm (464 cols = ~193 ns).  Inputs are pre-rounded to the
fp32r grid on the host; measured scale-rel error vs the fp32 reference is
~1.5e-4.
"""

import os
import sys

import numpy as np

sys.path.insert(0, "/opt/trn_rl_repo")

import concourse.tile as tile
from concourse import bacc, mybir

N, CIN, H, W = 32, 128, 56, 56
COUT, KH, KW = 256, 3, 3
NCORES = 8
NPER = N // NCORES  # images per core
HP, WP = H + 3, W + 2  # padded spatial dims (1 top + 2 bottom, 1 left + 1 right)
FLAT = HP * WP  # 3422 padded pixels per image per cin
RB = 8  # output rows per PSUM chunk
NRB = H // RB  # 7 row-blocks per image
CHUNK = RB * WP  # 464 <= 512 fp32 PSUM bank limit
NTAP = KH * KW

# input tile split: xa covers padded rows [0, 35) for rbs 0-3,
# xb covers padded rows [32, 59) for rbs 4-6
XA_ROWS = 35
XB_ROW0 = 32
XA_LEN = XA_ROWS * WP  # 2030
XB_LEN = (HP - XB_ROW0) * WP  # 27*58 = 1566

MM_MODE = os.environ.get("CONV_MM_MODE", "bf16")
NWARM = 48  # dummy warm-up matmuls (N=64) to trip the HAM clock gate early

_CACHE = {}


def _build(mm_mode):
    f32 = mybir.dt.float32
    in_dt = {
        "fp32": f32,
        "fp32r": mybir.dt.float32r,
        "bf16": mybir.dt.bfloat16,
    }[mm_mode]

    nc = bacc.Bacc(None, target_bir_lowering=False)
    xp_d = nc.declare_dram_parameter("xp", [NPER, CIN, FLAT], in_dt, isOutput=False)
    w_d = nc.declare_dram_parameter("w", [CIN, NTAP, COUT], in_dt, isOutput=False)
    b_d = nc.declare_dram_parameter("b", [CIN, 2], f32, isOutput=False)
    # padded output: [img, cout, rb, 8*58]; garbage cols sliced on host
    y_d = nc.declare_dram_parameter("y", [NPER, COUT, NRB, CHUNK], f32, isOutput=True)

    with tile.TileContext(nc) as tc:
        with (
            tc.tile_pool(name="xa", bufs=2) as xapool,
            tc.tile_pool(name="xb", bufs=2) as xbpool,
            tc.tile_pool(name="wgt", bufs=1) as wpool,
            tc.tile_pool(name="bias", bufs=1) as bpool,
            tc.tile_pool(name="out", bufs=8) as opool,
            tc.tile_pool(name="ps", bufs=8, space="PSUM") as pspool,
        ):
            # one tile per tap so an LDW only waits on its own tap's DMA
            w_taps = []
            for tap in range(NTAP):
                wt = wpool.tile([CIN, COUT], in_dt, tag=f"w{tap}")
                nc.scalar.dma_start(out=wt[:], in_=w_d[:, tap, :])
                w_taps.append(wt)
            b_sb = bpool.tile([CIN, 2], f32)
            nc.scalar.dma_start(out=b_sb[:], in_=b_d[:, :])

            # Warm-up: the PE clock gate (HAM) only opens to 2.4 GHz after
            # ~3.4us of sustained activity.  Issue dummy matmuls on scratch
            # SBUF while the first input DMA is in flight so the real
            # matmuls start (and stay) warm.
            warm_x = bpool.tile([CIN, 128], in_dt, tag="warm")
            nc.vector.memset(warm_x[:], 0)
            for wi in range(NWARM):
                wps = pspool.tile([128, 64], f32, name=f"warmps{wi}", tag="ps")
                nc.tensor.matmul(
                    wps[:], warm_x[:], warm_x[:, 0:64], start=True, stop=True
                )

            for i in range(NPER):
                xa = xapool.tile([CIN, XA_LEN], in_dt, tag="xa")
                nc.sync.dma_start(out=xa[:], in_=xp_d[i, :, 0:XA_LEN])
                xb = xbpool.tile([CIN, XB_LEN], in_dt, tag="xb")
                nc.sync.dma_start(
                    out=xb[:], in_=xp_d[i, :, XB_ROW0 * WP : XB_ROW0 * WP + XB_LEN]
                )
                for half in range(2):
                    ps_tiles = [
                        pspool.tile(
                            [128, CHUNK], f32, name=f"ps_{i}_{half}_{rb}", tag="ps"
                        )
                        for rb in range(NRB)
                    ]
                    for tap in range(NTAP):
                        kh, kw = divmod(tap, KW)
                        wsl = w_taps[tap][:, half * 128 : half * 128 + 128]
                        for rb in range(NRB):
                            off = (rb * RB + kh) * WP + kw
                            if rb < 4:
                                src = xa[:, off : off + CHUNK]
                            else:
                                o = off - XB_ROW0 * WP
                                src = xb[:, o : o + CHUNK]
                            nc.tensor.matmul(
                                ps_tiles[rb][:],
                                wsl,
                                src,
                                start=(tap == 0),
                                stop=(tap == NTAP - 1),
                            )
                    for rb in range(NRB):
                        ot = opool.tile([128, CHUNK], f32)
                        if rb % 2 == 0:
                            nc.vector.tensor_scalar_add(
                                ot[:], ps_tiles[rb][:], b_sb[:, half : half + 1]
                            )
                            dma_eng = nc.sync
                        else:
                            nc.scalar.activation(
                                ot[:],
                                ps_tiles[rb][:],
                                mybir.ActivationFunctionType.Identity,
                                bias=b_sb[:, half : half + 1],
                            )
                            dma_eng = nc.scalar
                        dma_eng.dma_start(
                            out=y_d[i, half * 128 : half * 128 + 128, rb, :],
                            in_=ot[:],
                        )
    nc.finalize()
    return nc


def get_nc(mm_mode=None):
    mm_mode = mm_mode or MM_MODE
    if mm_mode not in _CACHE:
        _CACHE[mm_mode] = _build(mm_mode)
    return _CACHE[mm_mode]


def _round_fp32r(a):
    """Round fp32 array to the fp32r grid (8-bit exp, 11-bit mantissa, top 20
    bits of the word) with round-to-nearest so the PE's truncation of the low
    12 bits lands on the nearest representable value."""
    u = np.ascontiguousarray(a, np.float32).view(np.uint32)
    u = u + 0x7FF + ((u >> 12) & 1)
    u &= np.uint32(0xFFFFF000)
    return u.view(np.float32)


def prep_inputs(x, weight, bias, mm_mode=None):
    """Host-side staging: zero-pad x to 59x58 and flatten, retile weights to
    [cin, tap, cout], split per-core input maps."""
    mm_mode = mm_mode or MM_MODE
    x = np.asarray(x, np.float32)
    weight = np.asarray(weight, np.float32)
    bias = np.asarray(bias, np.float32)

    xp = np.zeros((N, CIN, HP, WP), np.float32)
    xp[:, :, 1 : H + 1, 1 : W + 1] = x
    # [cout, cin, kh, kw] -> [cin, tap, cout]
    w_prep = np.ascontiguousarray(weight.transpose(1, 2, 3, 0).reshape(CIN, NTAP, COUT))
    if mm_mode == "bf16":
        import ml_dtypes

        xp = xp.astype(ml_dtypes.bfloat16)
        w_prep = w_prep.astype(ml_dtypes.bfloat16)
    elif mm_mode == "fp32r":
        xp = _round_fp32r(xp)
        w_prep = _round_fp32r(w_prep)
    xp = xp.reshape(N, CIN, FLAT)
    b_prep = np.ascontiguousarray(bias.reshape(2, 128).T.astype(np.float32))

    return [
        {
            "xp": np.ascontiguousarray(xp[c * NPER : (c + 1) * NPER]),
            "w": w_prep,
            "b": b_prep,
        }
        for c in range(NCORES)
    ]


def _unpad_output(y_pad):
    """[NPER, COUT, NRB, 464] padded rows -> [NPER, COUT, 56, 56]."""
    y = y_pad.reshape(NPER, COUT, NRB, RB, WP)[:, :, :, :, :W]
    return np.ascontiguousarray(y.reshape(NPER, COUT, H, W))


def kernel(x, weight, bias, mm_mode=None, trace=False, tmpdir=None):
    from concourse.bass_utils import run_bass_kernel_spmd

    nc = get_nc(mm_mode)
    in_maps = prep_inputs(x, weight, bias, mm_mode)
    res = run_bass_kernel_spmd(
        nc, in_maps, list(range(NCORES)), trace=trace, tmpdir=tmpdir
    )
    out = np.concatenate([_unpad_output(r["y"]) for r in res.results], axis=0)
    if trace:
        kernel.last_results = res
    return out
